# revision 19
# baseline (speedup 1.0000x reference)
"""Trainium2 Bass kernel for AffineNearestNeighborAttention (retrieval_knn).

Math (per row n):
  L[n,c]   = 2*x[n]@ctrs[c] - |ctrs[c]|^2          (= -dist^2 + |x|^2; row-const shift)
  tau[n]   = 16th largest of L[n,:]
  A[n,c]   = exp(L-tau) * (L >= tau)               (unnormalized top-16 softmax)
  W_eff    = A @ W_all                             (PE matmul, K=512, bf16)
             W_all cols 0..4095 (q,g): col q*64+g -> Wv[c,g,q]
             cols 4096..4159 = Ov[c,q], cols 4160..4163 = 1.0 (rowsum)
  out[n,q] = (sum_g x[n,g] * W_eff[n,(q,g)] + Ov_eff[n,q]) / rowsum(A)

Sharding: data-parallel over rows across 8 NeuronCores; ctrs/Wv/Ov replicated.
"""

import numpy as np

N, D, C, DO, K = 16384, 64, 512, 64, 16
NCORES = 8
NS = N // NCORES          # 2048 rows per core
NT = NS // 128            # 16 row-tiles per core
GP = D * DO               # 4096 (q,g) block
NW = GP + DO + 4          # 4164: + Ov cols + 4 ones-columns
G1 = D + 1                # 65

_CACHE = {}


def _build_program():
    import concourse.bass as bass
    import concourse.mybir as mybir
    from concourse import bacc
    from concourse.tile import TileContext
    from concourse.masks import make_identity
    from concourse.bass import ts

    f32 = mybir.dt.float32
    f32r = mybir.dt.float32r
    bf16 = mybir.dt.bfloat16
    AF = mybir.ActivationFunctionType
    ALU = mybir.AluOpType

    nc = bacc.Bacc("TRN2", target_bir_lowering=False, debug=False,
                   num_devices=NCORES)

    x_d = nc.dram_tensor("x", [NS, D], f32, kind="ExternalInput")
    ctrs_d = nc.dram_tensor("ctrs", [C, D], f32, kind="ExternalInput")
    wv_d = nc.dram_tensor("Wv", [C, D, DO], f32, kind="ExternalInput")
    ov_d = nc.dram_tensor("Ov", [C, DO], f32, kind="ExternalInput")
    out_d = nc.dram_tensor("out", [NS, DO], f32, kind="ExternalOutput")

    with TileContext(nc) as tc:
        with (
            tc.tile_pool(name="persist", bufs=1) as persist,
            tc.tile_pool(name="l_ps", bufs=2, space="PSUM") as l_ps,
            tc.tile_pool(name="a_ps", bufs=2, space="PSUM") as a_ps,
            tc.tile_pool(name="w_ps", bufs=4, space="PSUM") as w_ps,
            tc.tile_pool(name="small", bufs=4) as small,
            tc.tile_pool(name="l2p", bufs=2) as l2p,
            tc.tile_pool(name="lsp", bufs=2) as lsp,
            tc.tile_pool(name="ap_", bufs=2) as ap_,
            tc.tile_pool(name="amp", bufs=2) as amp,
            tc.tile_pool(name="atp", bufs=4) as atp,
            tc.tile_pool(name="w2d", bufs=4) as w2dp,
            tc.tile_pool(name="w3p", bufs=4) as w3p,
            tc.tile_pool(name="redp", bufs=3) as redp,
            tc.tile_pool(name="outp", bufs=4) as outp,
        ):
            # ---------- persistent SBUF ----------
            xp = persist.tile([128, NT * D], f32)          # x rows (f32, for einsum#2)
            xpb = persist.tile([128, NT * D], bf16)        # bf16 copy for gpsimd mults
            xqg = persist.tile([128, NT * 512], bf16)      # x repeated 8x over q (bf16)
            xqgF = persist.tile([128, NT * 512], f32)      # x repeated 8x over q (f32)
            xT = persist.tile([128, NS], f32r)             # rows 0-63 x^T, row 64 ones
            R = persist.tile([128, C], f32r)               # rows 0-63: 2*ctrs^T, row 64: -c2
            W_all = persist.tile([128, 4 * NW], bf16)      # [c-part, kc, (q,g)+Ov+ones]
            ident = persist.tile([128, 128], f32)
            identb = persist.tile([128, 128], bf16)
            ones_v = persist.tile([128, 1], f32)
            sq = persist.tile([128, C], f32)
            ctr_l = persist.tile([128, 4 * D], f32)

            make_identity(nc, ident)
            nc.gpsimd.memset(ones_v, 1.0)
            nc.gpsimd.memset(identb, 0.0)
            nc.scalar.copy(identb, ident)

            xp3 = xp.rearrange("a (t g) -> a t g", t=NT)
            xpb3 = xpb.rearrange("a (t g) -> a t g", t=NT)
            nc.sync.dma_start(xp3,
                              x_d.ap().rearrange("(t p) g -> p t g", p=128))
            nc.scalar.copy(xpb, xp)
            xqg4 = xqg.rearrange("a (t q g) -> a t q g", t=NT, q=8)
            xqgF4 = xqgF.rearrange("a (t q g) -> a t q g", t=NT, q=8)
            nc.scalar.copy(xqg4[:, :, 0, :], xpb3)
            nc.vector.tensor_copy(xqgF4[:, :, 0, :], xp3)
            for rep in (1, 2, 4):
                nc.scalar.copy(xqg4[:, :, rep:2 * rep, :],
                               xqg4[:, :, 0:rep, :])
                nc.vector.tensor_copy(xqgF4[:, :, rep:2 * rep, :],
                                      xqgF4[:, :, 0:rep, :])
            nc.sync.dma_start(ctr_l.rearrange("a (kc g) -> a kc g", kc=4),
                              ctrs_d.ap().rearrange("(kc p) g -> p kc g", p=128))

            # W_all: load g-major into temp, cast-copy to (q,g)-major bf16
            W_all4 = W_all.rearrange("a (kc w) -> a kc w", kc=4)
            for kc in range(4):
                wtmp = w2dp.tile([128, NW], f32, tag="Wt")
                nc.sync.dma_start(
                    wtmp[:, 0:GP],
                    wv_d.ap().rearrange("(kc p) g q -> p kc (g q)",
                                        p=128)[:, kc, :])
                nc.sync.dma_start(
                    wtmp[:, GP:GP + DO],
                    ov_d.ap().rearrange("(kc p) q -> p kc q", p=128)[:, kc, :])
                # transpose free dims: (g,q) g-major -> (q,g), cast bf16
                nc.scalar.copy(
                    W_all4[:, kc, 0:GP].rearrange("a (q g) -> a g q", q=DO),
                    wtmp[:, 0:GP].rearrange("a (g q) -> a g q", g=D))
                nc.scalar.copy(W_all4[:, kc, GP:GP + DO], wtmp[:, GP:GP + DO])
                nc.vector.memset(wtmp[:, GP + DO:NW], 1.0)
                nc.scalar.copy(W_all4[:, kc, GP + DO:NW], wtmp[:, GP + DO:NW])

            # ---------- R = [2*ctrs^T ; -c2] (f32r) ----------
            for kc in range(4):
                pt = w_ps.tile([128, 512], f32, tag="wp")
                nc.tensor.transpose(pt[0:D, 0:128], ctr_l[:, ts(kc, D)], ident)
                nc.scalar.mul(R[0:D, ts(kc, 128)], pt[0:D, 0:128], 2.0)
            nc.scalar.square(sq[0:D, :], R[0:D, :])        # (2c)^2
            c2p = l_ps.tile([128, C], f32, tag="Lp")
            nc.tensor.matmul(c2p[0:1, :], ones_v[0:D, :], sq[0:D, :],
                             start=True, stop=True)
            nc.scalar.mul(R[D:D + 1, :], c2p[0:1, :], -0.25)

            # ---------- x^T (PE transposes; ones row 64) ----------
            nc.gpsimd.memset(xT[D:D + 1, :].bitcast(f32), 1.0)
            for t in range(NT):
                pt = w_ps.tile([128, 512], f32, tag="wp")
                nc.tensor.transpose(pt[0:D, 0:128], xp3[:, t, :], ident)
                nc.scalar.copy(xT[0:D, ts(t, 128)], pt[0:D, 0:128])

            xqg3 = xqg.rearrange("a (t w) -> a t w", t=NT)
            xqgF3 = xqgF.rearrange("a (t w) -> a t w", t=NT)

            def _einsum2(t, chunk, wp, red):
                # einsum#2 over one 512-col psum chunk: 8 q's, 64 g's
                w3 = w3p.tile([128, 512], bf16, tag="w3")
                if chunk < 6:
                    # ACT drains psum (cast bf16), GPSIMD multiplies
                    # (both mult operands contiguous bf16)
                    w2d = w2dp.tile([128, 512], bf16, tag="w2d")
                    nc.scalar.copy(w2d, wp)
                    nc.gpsimd.tensor_mul(w3, w2d, xqg3[:, t, :])
                else:
                    # DVE multiplies straight from PSUM (f32 x f32)
                    nc.vector.tensor_mul(w3, wp, xqgF3[:, t, :])
                with nc.allow_low_precision("bf16 reduce, f32 accum"):
                    nc.vector.tensor_reduce(
                        red[:, chunk * 8:chunk * 8 + 8],
                        w3.rearrange("a (q g) -> a q g", q=8),
                        axis=mybir.AxisListType.X, op=ALU.add)

            # ---------- per row-tile pipeline ----------
            def front(t):
                # logits L = x' @ R  -> PSUM [128, 512]  (f32r, 1cyc/col)
                Lp = l_ps.tile([128, C], f32, tag="Lp")
                nc.tensor.matmul(Lp, xT[0:D + 1, ts(t, 128)], R[0:D + 1, :],
                                 start=True, stop=True)

                # copy logits to SBUF (max/match_replace are SBUF-only ops)
                Ls = lsp.tile([128, C], f32)
                nc.scalar.copy(Ls, Lp)

                # 16th-largest threshold tau per row
                m1 = small.tile([128, 8], f32, tag="m1")
                nc.vector.max(out=m1, in_=Ls)
                L2 = l2p.tile([128, C], f32)
                nc.vector.match_replace(out=L2, in_to_replace=m1,
                                        in_values=Ls, imm_value=-3.0e38)
                m2 = small.tile([128, 8], f32, tag="m2")
                nc.vector.max(out=m2, in_=L2)
                ntau = small.tile([128, 1], f32, tag="ntau")
                nc.scalar.mul(ntau, m2[:, 7:8], -1.0)

                # A = exp(L - tau) masked to top-16:
                #   Ae  = exp(L - tau)        (all 512)
                #   Ae3 = exp(L3 - tau)       (L3 = L with top-16 removed)
                #   Am  = Ae - Ae3            (exact zeros off the top-16)
                Ae = ap_.tile([128, C], f32)
                nc.scalar.activation(Ae, Ls, AF.Exp, bias=ntau, scale=1.0)
                L3 = amp.tile([128, C], f32, tag="L3")
                nc.vector.match_replace(out=L3, in_to_replace=m2,
                                        in_values=L2, imm_value=-3.0e38)
                Ae3 = amp.tile([128, C], f32, tag="Ae3")
                nc.scalar.activation(Ae3, L3, AF.Exp, bias=ntau, scale=1.0)
                Am = amp.tile([128, C], bf16, tag="Am")
                nc.gpsimd.tensor_sub(Am, Ae, Ae3)

                # A^T via PE transposes (bf16) -> one psum tile, one ACT copy
                AT = atp.tile([128, 4 * 128], bf16)
                ptA = a_ps.tile([128, 512], bf16, tag="ptAb")
                for kc in range(4):
                    nc.tensor.transpose(ptA[:, ts(kc, 128)], Am[:, ts(kc, 128)],
                                        identb)
                nc.scalar.copy(AT, ptA)
                return AT

            def back(t, AT):
                # einsum#1: W_eff = A @ W_all  (bf16, K=512), 8 psum chunks
                red = redp.tile([128, DO], bf16)
                for chunk in range(8):
                    wp = w_ps.tile([128, 512], f32, tag="wp")
                    off = chunk * 512
                    for kc in range(4):
                        nc.tensor.matmul(
                            wp, AT[:, ts(kc, 128)],
                            W_all4[:, kc, off:off + 512],
                            start=(kc == 0), stop=(kc == 3))
                    _einsum2(t, chunk, wp, red)


                # tail: Ov + rowsum cols (4096..4163)
                wpt = w_ps.tile([128, 512], f32, tag="wp")
                for kc in range(4):
                    nc.tensor.matmul(wpt[:, 0:NW - GP],
                                     AT[:, ts(kc, 128)],
                                     W_all4[:, kc, GP:NW],
                                     start=(kc == 0), stop=(kc == 3))
                W2t = outp.tile([128, NW - GP], f32, tag="W2t")
                nc.scalar.copy(W2t, wpt[:, 0:NW - GP])

                rs = small.tile([128, 1], f32, tag="rs")
                nc.vector.reciprocal(rs, W2t[:, DO:DO + 1])
                o_main = outp.tile([128, DO], f32, tag="om")
                nc.gpsimd.tensor_add(o_main, red, W2t[:, 0:DO])
                o3 = outp.tile([128, DO], f32, tag="o3")
                nc.gpsimd.tensor_scalar_mul(o3, o_main, rs)
                nc.sync.dma_start(out_d[ts(t, 128), :], o3)

            pend = []
            for t in range(NT):
                pend.append((t, front(t)))
                if len(pend) > 2:
                    back(*pend.pop(0))
            for item in pend:
                back(*item)

    nc.compile()
    return nc


def kernel(x, ctrs, Wv, Ov, k):
    from concourse.bass_utils import run_bass_kernel_spmd

    assert int(k) == K
    x = np.ascontiguousarray(np.asarray(x, dtype=np.float32))
    ctrs = np.ascontiguousarray(np.asarray(ctrs, dtype=np.float32))
    Wv = np.ascontiguousarray(np.asarray(Wv, dtype=np.float32))
    Ov = np.ascontiguousarray(np.asarray(Ov, dtype=np.float32))

    if "nc" not in _CACHE:
        _CACHE["nc"] = _build_program()
    nc = _CACHE["nc"]

    in_maps = [
        {"x": x[i * NS:(i + 1) * NS], "ctrs": ctrs, "Wv": Wv, "Ov": Ov}
        for i in range(NCORES)
    ]
    res = run_bass_kernel_spmd(nc, in_maps, core_ids=list(range(NCORES)))
    out = np.concatenate([res.results[i]["out"] for i in range(NCORES)], axis=0)
    return out.astype(np.float32)


# revision 20
# speedup vs baseline: 1.0303x; 1.0303x over previous
"""Trainium2 Bass kernel for AffineNearestNeighborAttention (retrieval_knn).

Math (per row n):
  L[n,c]   = 2*x[n]@ctrs[c] - |ctrs[c]|^2          (= -dist^2 + |x|^2; row-const shift)
  tau[n]   = 16th largest of L[n,:]
  A[n,c]   = exp(L-tau) * (L >= tau)               (unnormalized top-16 softmax)
  W_eff    = A @ W_all                             (PE matmul, K=512, bf16)
             W_all cols 0..4095 (q,g): col q*64+g -> Wv[c,g,q]
             cols 4096..4159 = Ov[c,q], cols 4160..4163 = 1.0 (rowsum)
  out[n,q] = (sum_g x[n,g] * W_eff[n,(q,g)] + Ov_eff[n,q]) / rowsum(A)

Sharding: data-parallel over rows across 8 NeuronCores; ctrs/Wv/Ov replicated.
"""

import numpy as np

N, D, C, DO, K = 16384, 64, 512, 64, 16
NCORES = 8
NS = N // NCORES          # 2048 rows per core
NT = NS // 128            # 16 row-tiles per core
GP = D * DO               # 4096 (q,g) block
NW = GP + DO + 4          # 4164: + Ov cols + 4 ones-columns
G1 = D + 1                # 65

_CACHE = {}


def _build_program():
    import concourse.bass as bass
    import concourse.mybir as mybir
    from concourse import bacc
    from concourse.tile import TileContext
    from concourse.masks import make_identity
    from concourse.bass import ts

    f32 = mybir.dt.float32
    f32r = mybir.dt.float32r
    bf16 = mybir.dt.bfloat16
    AF = mybir.ActivationFunctionType
    ALU = mybir.AluOpType

    nc = bacc.Bacc("TRN2", target_bir_lowering=False, debug=False,
                   num_devices=NCORES)

    x_d = nc.dram_tensor("x", [NS, D], f32, kind="ExternalInput")
    ctrs_d = nc.dram_tensor("ctrs", [C, D], f32, kind="ExternalInput")
    wv_d = nc.dram_tensor("Wv", [C, D, DO], f32, kind="ExternalInput")
    ov_d = nc.dram_tensor("Ov", [C, DO], f32, kind="ExternalInput")
    out_d = nc.dram_tensor("out", [NS, DO], f32, kind="ExternalOutput")

    with TileContext(nc) as tc:
        with (
            tc.tile_pool(name="persist", bufs=1) as persist,
            tc.tile_pool(name="l_ps", bufs=2, space="PSUM") as l_ps,
            tc.tile_pool(name="a_ps", bufs=2, space="PSUM") as a_ps,
            tc.tile_pool(name="w_ps", bufs=4, space="PSUM") as w_ps,
            tc.tile_pool(name="small", bufs=4) as small,
            tc.tile_pool(name="l2p", bufs=2) as l2p,
            tc.tile_pool(name="lsp", bufs=2) as lsp,
            tc.tile_pool(name="ap_", bufs=2) as ap_,
            tc.tile_pool(name="amp", bufs=2) as amp,
            tc.tile_pool(name="atp", bufs=4) as atp,
            tc.tile_pool(name="w2d", bufs=4) as w2dp,
            tc.tile_pool(name="w3p", bufs=4) as w3p,
            tc.tile_pool(name="redp", bufs=3) as redp,
            tc.tile_pool(name="outp", bufs=4) as outp,
        ):
            # ---------- persistent SBUF ----------
            xp = persist.tile([128, NT * D], f32)          # x rows (f32, for einsum#2)
            xpb = persist.tile([128, NT * D], bf16)        # bf16 copy for gpsimd mults
            xqg = persist.tile([128, NT * 512], bf16)      # x repeated 8x over q (bf16)
            xqgF = persist.tile([128, NT * 512], f32)      # x repeated 8x over q (f32)
            xT = persist.tile([128, NS], f32r)             # rows 0-63 x^T, row 64 ones
            R = persist.tile([128, C], f32r)               # rows 0-63: 2*ctrs^T, row 64: -c2
            W_all = persist.tile([128, 4 * NW], bf16)      # [c-part, kc, (q,g)+Ov+ones]
            ident = persist.tile([128, 128], f32)
            identb = persist.tile([128, 128], bf16)
            ones_v = persist.tile([128, 1], f32)
            sq = persist.tile([128, C], f32)
            ctr_l = persist.tile([128, 4 * D], f32)

            make_identity(nc, ident)
            nc.gpsimd.memset(ones_v, 1.0)
            nc.gpsimd.memset(identb, 0.0)
            nc.scalar.copy(identb, ident)

            xp3 = xp.rearrange("a (t g) -> a t g", t=NT)
            xpb3 = xpb.rearrange("a (t g) -> a t g", t=NT)
            nc.sync.dma_start(xp3,
                              x_d.ap().rearrange("(t p) g -> p t g", p=128))
            nc.scalar.copy(xpb, xp)
            xqg4 = xqg.rearrange("a (t q g) -> a t q g", t=NT, q=8)
            xqgF4 = xqgF.rearrange("a (t q g) -> a t q g", t=NT, q=8)
            nc.scalar.copy(xqg4[:, :, 0, :], xpb3)
            nc.vector.tensor_copy(xqgF4[:, :, 0, :], xp3)
            for rep in (1, 2, 4):
                nc.scalar.copy(xqg4[:, :, rep:2 * rep, :],
                               xqg4[:, :, 0:rep, :])
                nc.vector.tensor_copy(xqgF4[:, :, rep:2 * rep, :],
                                      xqgF4[:, :, 0:rep, :])
            nc.sync.dma_start(ctr_l.rearrange("a (kc g) -> a kc g", kc=4),
                              ctrs_d.ap().rearrange("(kc p) g -> p kc g", p=128))

            # W_all: load g-major into temp, cast-copy to (q,g)-major bf16
            W_all4 = W_all.rearrange("a (kc w) -> a kc w", kc=4)
            for kc in range(4):
                wtmp = w2dp.tile([128, NW], f32, tag="Wt")
                nc.sync.dma_start(
                    wtmp[:, 0:GP],
                    wv_d.ap().rearrange("(kc p) g q -> p kc (g q)",
                                        p=128)[:, kc, :])
                nc.sync.dma_start(
                    wtmp[:, GP:GP + DO],
                    ov_d.ap().rearrange("(kc p) q -> p kc q", p=128)[:, kc, :])
                # transpose free dims: (g,q) g-major -> (q,g), cast bf16
                nc.scalar.copy(
                    W_all4[:, kc, 0:GP].rearrange("a (q g) -> a g q", q=DO),
                    wtmp[:, 0:GP].rearrange("a (g q) -> a g q", g=D))
                nc.scalar.copy(W_all4[:, kc, GP:GP + DO], wtmp[:, GP:GP + DO])
                nc.vector.memset(wtmp[:, GP + DO:NW], 1.0)
                nc.scalar.copy(W_all4[:, kc, GP + DO:NW], wtmp[:, GP + DO:NW])

            # ---------- R = [2*ctrs^T ; -c2] (f32r) ----------
            for kc in range(4):
                pt = w_ps.tile([128, 512], f32, tag="wp")
                nc.tensor.transpose(pt[0:D, 0:128], ctr_l[:, ts(kc, D)], ident)
                nc.scalar.mul(R[0:D, ts(kc, 128)], pt[0:D, 0:128], 2.0)
            nc.scalar.square(sq[0:D, :], R[0:D, :])        # (2c)^2
            c2p = l_ps.tile([128, C], f32, tag="Lp")
            nc.tensor.matmul(c2p[0:1, :], ones_v[0:D, :], sq[0:D, :],
                             start=True, stop=True)
            nc.scalar.mul(R[D:D + 1, :], c2p[0:1, :], -0.25)

            # ---------- x^T (PE transposes; ones row 64) ----------
            nc.gpsimd.memset(xT[D:D + 1, :].bitcast(f32), 1.0)
            for t in range(NT):
                pt = w_ps.tile([128, 512], f32, tag="wp")
                nc.tensor.transpose(pt[0:D, 0:128], xp3[:, t, :], ident)
                nc.scalar.copy(xT[0:D, ts(t, 128)], pt[0:D, 0:128])

            xqg3 = xqg.rearrange("a (t w) -> a t w", t=NT)
            xqgF3 = xqgF.rearrange("a (t w) -> a t w", t=NT)

            def _einsum2(t, chunk, wp, red):
                # einsum#2 over one 512-col psum chunk: 8 q's, 64 g's
                w3 = w3p.tile([128, 512], bf16, tag="w3")
                if chunk < 5:
                    # ACT drains psum (cast bf16), GPSIMD multiplies
                    # (both mult operands contiguous bf16)
                    w2d = w2dp.tile([128, 512], bf16, tag="w2d")
                    nc.scalar.copy(w2d, wp)
                    nc.gpsimd.tensor_mul(w3, w2d, xqg3[:, t, :])
                else:
                    # DVE multiplies straight from PSUM (f32 x f32)
                    nc.vector.tensor_mul(w3, wp, xqgF3[:, t, :])
                with nc.allow_low_precision("bf16 reduce, f32 accum"):
                    nc.vector.tensor_reduce(
                        red[:, chunk * 8:chunk * 8 + 8],
                        w3.rearrange("a (q g) -> a q g", q=8),
                        axis=mybir.AxisListType.X, op=ALU.add)

            # ---------- per row-tile pipeline ----------
            def front(t):
                # logits L = x' @ R  -> PSUM [128, 512]  (f32r, 1cyc/col)
                Lp = l_ps.tile([128, C], f32, tag="Lp")
                nc.tensor.matmul(Lp, xT[0:D + 1, ts(t, 128)], R[0:D + 1, :],
                                 start=True, stop=True)

                # copy logits to SBUF (max/match_replace are SBUF-only ops)
                Ls = lsp.tile([128, C], f32)
                nc.scalar.copy(Ls, Lp)

                # 16th-largest threshold tau per row
                m1 = small.tile([128, 8], f32, tag="m1")
                nc.vector.max(out=m1, in_=Ls)
                L2 = l2p.tile([128, C], f32)
                nc.vector.match_replace(out=L2, in_to_replace=m1,
                                        in_values=Ls, imm_value=-3.0e38)
                m2 = small.tile([128, 8], f32, tag="m2")
                nc.vector.max(out=m2, in_=L2)
                ntau = small.tile([128, 1], f32, tag="ntau")
                nc.scalar.mul(ntau, m2[:, 7:8], -1.0)

                # A = exp(L - tau) masked to top-16:
                #   Ae  = exp(L - tau)        (all 512)
                #   Ae3 = exp(L3 - tau)       (L3 = L with top-16 removed)
                #   Am  = Ae - Ae3            (exact zeros off the top-16)
                Ae = ap_.tile([128, C], f32)
                nc.scalar.activation(Ae, Ls, AF.Exp, bias=ntau, scale=1.0)
                L3 = amp.tile([128, C], f32, tag="L3")
                nc.vector.match_replace(out=L3, in_to_replace=m2,
                                        in_values=L2, imm_value=-3.0e38)
                Ae3 = amp.tile([128, C], f32, tag="Ae3")
                nc.scalar.activation(Ae3, L3, AF.Exp, bias=ntau, scale=1.0)
                Am = amp.tile([128, C], bf16, tag="Am")
                nc.gpsimd.tensor_sub(Am, Ae, Ae3)

                # A^T via PE transposes (bf16) -> one psum tile, one ACT copy
                AT = atp.tile([128, 4 * 128], bf16)
                ptA = a_ps.tile([128, 512], bf16, tag="ptAb")
                for kc in range(4):
                    nc.tensor.transpose(ptA[:, ts(kc, 128)], Am[:, ts(kc, 128)],
                                        identb)
                nc.scalar.copy(AT, ptA)
                return AT

            def back(t, AT):
                # einsum#1: W_eff = A @ W_all  (bf16, K=512), 8 psum chunks
                red = redp.tile([128, DO], bf16)
                for chunk in range(8):
                    wp = w_ps.tile([128, 512], f32, tag="wp")
                    off = chunk * 512
                    for kc in range(4):
                        nc.tensor.matmul(
                            wp, AT[:, ts(kc, 128)],
                            W_all4[:, kc, off:off + 512],
                            start=(kc == 0), stop=(kc == 3))
                    _einsum2(t, chunk, wp, red)


                # tail: Ov + rowsum cols (4096..4163)
                wpt = w_ps.tile([128, 512], f32, tag="wp")
                for kc in range(4):
                    nc.tensor.matmul(wpt[:, 0:NW - GP],
                                     AT[:, ts(kc, 128)],
                                     W_all4[:, kc, GP:NW],
                                     start=(kc == 0), stop=(kc == 3))
                W2t = outp.tile([128, NW - GP], f32, tag="W2t")
                nc.scalar.copy(W2t, wpt[:, 0:NW - GP])

                rs = small.tile([128, 1], f32, tag="rs")
                nc.vector.reciprocal(rs, W2t[:, DO:DO + 1])
                o_main = outp.tile([128, DO], f32, tag="om")
                nc.gpsimd.tensor_add(o_main, red, W2t[:, 0:DO])
                o3 = outp.tile([128, DO], f32, tag="o3")
                nc.scalar.activation(o3, o_main, AF.Copy, scale=rs)
                nc.sync.dma_start(out_d[ts(t, 128), :], o3)

            pend = []
            for t in range(NT):
                pend.append((t, front(t)))
                if len(pend) > 2:
                    back(*pend.pop(0))
            for item in pend:
                back(*item)

    nc.compile()
    return nc


def kernel(x, ctrs, Wv, Ov, k):
    from concourse.bass_utils import run_bass_kernel_spmd

    assert int(k) == K
    x = np.ascontiguousarray(np.asarray(x, dtype=np.float32))
    ctrs = np.ascontiguousarray(np.asarray(ctrs, dtype=np.float32))
    Wv = np.ascontiguousarray(np.asarray(Wv, dtype=np.float32))
    Ov = np.ascontiguousarray(np.asarray(Ov, dtype=np.float32))

    if "nc" not in _CACHE:
        _CACHE["nc"] = _build_program()
    nc = _CACHE["nc"]

    in_maps = [
        {"x": x[i * NS:(i + 1) * NS], "ctrs": ctrs, "Wv": Wv, "Ov": Ov}
        for i in range(NCORES)
    ]
    res = run_bass_kernel_spmd(nc, in_maps, core_ids=list(range(NCORES)))
    out = np.concatenate([res.results[i]["out"] for i in range(NCORES)], axis=0)
    return out.astype(np.float32)


# revision 21
# speedup vs baseline: 1.0561x; 1.0250x over previous
"""Trainium2 Bass kernel for AffineNearestNeighborAttention (retrieval_knn).

Math (per row n):
  L[n,c]   = 2*x[n]@ctrs[c] - |ctrs[c]|^2          (= -dist^2 + |x|^2; row-const shift)
  tau[n]   = 16th largest of L[n,:]
  A[n,c]   = exp(L-tau) * (L >= tau)               (unnormalized top-16 softmax)
  W_eff    = A @ W_all                             (PE matmul, K=512, bf16)
             W_all cols 0..4095 (q,g): col q*64+g -> Wv[c,g,q]
             cols 4096..4159 = Ov[c,q], cols 4160..4163 = 1.0 (rowsum)
  out[n,q] = (sum_g x[n,g] * W_eff[n,(q,g)] + Ov_eff[n,q]) / rowsum(A)

Sharding: data-parallel over rows across 8 NeuronCores; ctrs/Wv/Ov replicated.
"""

import numpy as np

N, D, C, DO, K = 16384, 64, 512, 64, 16
NCORES = 8
NS = N // NCORES          # 2048 rows per core
NT = NS // 128            # 16 row-tiles per core
GP = D * DO               # 4096 (q,g) block
NW = GP + DO + 4          # 4164: + Ov cols + 4 ones-columns
G1 = D + 1                # 65

_CACHE = {}


def _build_program():
    import concourse.bass as bass
    import concourse.mybir as mybir
    from concourse import bacc
    from concourse.tile import TileContext
    from concourse.masks import make_identity
    from concourse.bass import ts

    f32 = mybir.dt.float32
    f32r = mybir.dt.float32r
    bf16 = mybir.dt.bfloat16
    AF = mybir.ActivationFunctionType
    ALU = mybir.AluOpType

    nc = bacc.Bacc("TRN2", target_bir_lowering=False, debug=False,
                   num_devices=NCORES)

    x_d = nc.dram_tensor("x", [NS, D], f32, kind="ExternalInput")
    ctrs_d = nc.dram_tensor("ctrs", [C, D], f32, kind="ExternalInput")
    wv_d = nc.dram_tensor("Wv", [C, D, DO], f32, kind="ExternalInput")
    ov_d = nc.dram_tensor("Ov", [C, DO], f32, kind="ExternalInput")
    out_d = nc.dram_tensor("out", [NS, DO], f32, kind="ExternalOutput")

    with TileContext(nc) as tc:
        with (
            tc.tile_pool(name="persist", bufs=1) as persist,
            tc.tile_pool(name="l_ps", bufs=2, space="PSUM") as l_ps,
            tc.tile_pool(name="a_ps", bufs=2, space="PSUM") as a_ps,
            tc.tile_pool(name="w_ps", bufs=4, space="PSUM") as w_ps,
            tc.tile_pool(name="small", bufs=4) as small,
            tc.tile_pool(name="l2p", bufs=2) as l2p,
            tc.tile_pool(name="lsp", bufs=2) as lsp,
            tc.tile_pool(name="ap_", bufs=2) as ap_,
            tc.tile_pool(name="amp", bufs=2) as amp,
            tc.tile_pool(name="atp", bufs=4) as atp,
            tc.tile_pool(name="w2d", bufs=4) as w2dp,
            tc.tile_pool(name="w3p", bufs=4) as w3p,
            tc.tile_pool(name="redp", bufs=3) as redp,
            tc.tile_pool(name="outp", bufs=4) as outp,
        ):
            # ---------- persistent SBUF ----------
            xp = persist.tile([128, NT * D], f32)          # x rows (f32, for einsum#2)
            xpb = persist.tile([128, NT * D], bf16)        # bf16 copy for gpsimd mults
            xT = persist.tile([128, NS], f32r)             # rows 0-63 x^T, row 64 ones
            R = persist.tile([128, C], f32r)               # rows 0-63: 2*ctrs^T, row 64: -c2
            W_all = persist.tile([128, 4 * NW], bf16)      # [c-part, kc, (q,g)+Ov+ones]
            ident = persist.tile([128, 128], f32)
            identb = persist.tile([128, 128], bf16)
            ones_v = persist.tile([128, 1], f32)
            sq = persist.tile([128, C], f32)
            ctr_l = persist.tile([128, 4 * D], f32)

            make_identity(nc, ident)
            nc.gpsimd.memset(ones_v, 1.0)
            nc.gpsimd.memset(identb, 0.0)
            nc.scalar.copy(identb, ident)

            xp3 = xp.rearrange("a (t g) -> a t g", t=NT)
            xpb3 = xpb.rearrange("a (t g) -> a t g", t=NT)
            nc.sync.dma_start(xp3,
                              x_d.ap().rearrange("(t p) g -> p t g", p=128))
            nc.scalar.copy(xpb, xp)
            nc.sync.dma_start(ctr_l.rearrange("a (kc g) -> a kc g", kc=4),
                              ctrs_d.ap().rearrange("(kc p) g -> p kc g", p=128))

            # W_all: load g-major into temp, cast-copy to (q,g)-major bf16
            W_all4 = W_all.rearrange("a (kc w) -> a kc w", kc=4)
            for kc in range(4):
                wtmp = w2dp.tile([128, NW], f32, tag="Wt")
                nc.sync.dma_start(
                    wtmp[:, 0:GP],
                    wv_d.ap().rearrange("(kc p) g q -> p kc (g q)",
                                        p=128)[:, kc, :])
                nc.sync.dma_start(
                    wtmp[:, GP:GP + DO],
                    ov_d.ap().rearrange("(kc p) q -> p kc q", p=128)[:, kc, :])
                # transpose free dims: (g,q) g-major -> (q,g), cast bf16
                nc.scalar.copy(
                    W_all4[:, kc, 0:GP].rearrange("a (q g) -> a g q", q=DO),
                    wtmp[:, 0:GP].rearrange("a (g q) -> a g q", g=D))
                nc.scalar.copy(W_all4[:, kc, GP:GP + DO], wtmp[:, GP:GP + DO])
                nc.vector.memset(wtmp[:, GP + DO:NW], 1.0)
                nc.scalar.copy(W_all4[:, kc, GP + DO:NW], wtmp[:, GP + DO:NW])

            # ---------- R = [2*ctrs^T ; -c2] (f32r) ----------
            for kc in range(4):
                pt = w_ps.tile([128, 512], f32, tag="wp")
                nc.tensor.transpose(pt[0:D, 0:128], ctr_l[:, ts(kc, D)], ident)
                nc.scalar.mul(R[0:D, ts(kc, 128)], pt[0:D, 0:128], 2.0)
            nc.scalar.square(sq[0:D, :], R[0:D, :])        # (2c)^2
            c2p = l_ps.tile([128, C], f32, tag="Lp")
            nc.tensor.matmul(c2p[0:1, :], ones_v[0:D, :], sq[0:D, :],
                             start=True, stop=True)
            nc.scalar.mul(R[D:D + 1, :], c2p[0:1, :], -0.25)

            # ---------- x^T (PE transposes; ones row 64) ----------
            nc.gpsimd.memset(xT[D:D + 1, :].bitcast(f32), 1.0)
            for t in range(NT):
                pt = w_ps.tile([128, 512], f32, tag="wp")
                nc.tensor.transpose(pt[0:D, 0:128], xp3[:, t, :], ident)
                nc.scalar.copy(xT[0:D, ts(t, 128)], pt[0:D, 0:128])

            def _einsum2(t, chunk, wp, red, xbf, xbb):
                # einsum#2 over one 512-col psum chunk: 8 q's, 64 g's
                w3 = w3p.tile([128, 512], bf16, tag="w3")
                w3v = w3.rearrange("a (q g) -> a q g", q=8)
                if chunk < 5:
                    # ACT drains psum (cast bf16), GPSIMD multiplies
                    w2d = w2dp.tile([128, 512], bf16, tag="w2d")
                    nc.scalar.copy(w2d, wp)
                    nc.gpsimd.tensor_mul(w3v,
                                         w2d.rearrange("a (q g) -> a q g",
                                                       q=8), xbb)
                else:
                    # DVE multiplies straight from PSUM
                    nc.vector.tensor_mul(w3v,
                                         wp.rearrange("a (q g) -> a q g",
                                                      q=8), xbf)
                with nc.allow_low_precision("bf16 reduce, f32 accum"):
                    nc.vector.tensor_reduce(
                        red[:, chunk * 8:chunk * 8 + 8], w3v,
                        axis=mybir.AxisListType.X, op=ALU.add)

            # ---------- per row-tile pipeline ----------
            def front(t):
                # logits L = x' @ R  -> PSUM [128, 512]  (f32r, 1cyc/col)
                Lp = l_ps.tile([128, C], f32, tag="Lp")
                nc.tensor.matmul(Lp, xT[0:D + 1, ts(t, 128)], R[0:D + 1, :],
                                 start=True, stop=True)

                # copy logits to SBUF (max/match_replace are SBUF-only ops)
                Ls = lsp.tile([128, C], f32)
                nc.scalar.copy(Ls, Lp)

                # 16th-largest threshold tau per row
                m1 = small.tile([128, 8], f32, tag="m1")
                nc.vector.max(out=m1, in_=Ls)
                L2 = l2p.tile([128, C], f32)
                nc.vector.match_replace(out=L2, in_to_replace=m1,
                                        in_values=Ls, imm_value=-3.0e38)
                m2 = small.tile([128, 8], f32, tag="m2")
                nc.vector.max(out=m2, in_=L2)
                ntau = small.tile([128, 1], f32, tag="ntau")
                nc.scalar.mul(ntau, m2[:, 7:8], -1.0)

                # A = exp(L - tau) masked to top-16:
                #   Ae  = exp(L - tau)        (all 512)
                #   Ae3 = exp(L3 - tau)       (L3 = L with top-16 removed)
                #   Am  = Ae - Ae3            (exact zeros off the top-16)
                Ae = ap_.tile([128, C], f32)
                nc.scalar.activation(Ae, Ls, AF.Exp, bias=ntau, scale=1.0)
                L3 = amp.tile([128, C], f32, tag="L3")
                nc.vector.match_replace(out=L3, in_to_replace=m2,
                                        in_values=L2, imm_value=-3.0e38)
                Ae3 = amp.tile([128, C], f32, tag="Ae3")
                nc.scalar.activation(Ae3, L3, AF.Exp, bias=ntau, scale=1.0)
                Am = amp.tile([128, C], bf16, tag="Am")
                nc.gpsimd.tensor_sub(Am, Ae, Ae3)

                # A^T via PE transposes (bf16) -> one psum tile, one ACT copy
                AT = atp.tile([128, 4 * 128], bf16)
                ptA = a_ps.tile([128, 512], bf16, tag="ptAb")
                for kc in range(4):
                    nc.tensor.transpose(ptA[:, ts(kc, 128)], Am[:, ts(kc, 128)],
                                        identb)
                nc.scalar.copy(AT, ptA)
                return AT

            def back(t, AT):
                # einsum#1: W_eff = A @ W_all  (bf16, K=512), 8 psum chunks
                red = redp.tile([128, DO], bf16)
                xq = xp3[:, t, :]
                xqb = xpb3[:, t, :]
                xbf = (xq.to_broadcast([128, D, 8])
                       .rearrange("a g q -> a q g"))
                xbb = (xqb.to_broadcast([128, D, 8])
                       .rearrange("a g q -> a q g"))
                for chunk in range(8):
                    wp = w_ps.tile([128, 512], f32, tag="wp")
                    off = chunk * 512
                    for kc in range(4):
                        nc.tensor.matmul(
                            wp, AT[:, ts(kc, 128)],
                            W_all4[:, kc, off:off + 512],
                            start=(kc == 0), stop=(kc == 3))
                    _einsum2(t, chunk, wp, red, xbf, xbb)


                # tail: Ov + rowsum cols (4096..4163)
                wpt = w_ps.tile([128, 512], f32, tag="wp")
                for kc in range(4):
                    nc.tensor.matmul(wpt[:, 0:NW - GP],
                                     AT[:, ts(kc, 128)],
                                     W_all4[:, kc, GP:NW],
                                     start=(kc == 0), stop=(kc == 3))
                W2t = outp.tile([128, NW - GP], f32, tag="W2t")
                nc.scalar.copy(W2t, wpt[:, 0:NW - GP])

                rs = small.tile([128, 1], f32, tag="rs")
                nc.vector.reciprocal(rs, W2t[:, DO:DO + 1])
                o_main = outp.tile([128, DO], f32, tag="om")
                nc.gpsimd.tensor_add(o_main, red, W2t[:, 0:DO])
                o3 = outp.tile([128, DO], f32, tag="o3")
                nc.gpsimd.tensor_scalar_mul(o3, o_main, rs)
                nc.sync.dma_start(out_d[ts(t, 128), :], o3)

            pend = []
            for t in range(NT):
                pend.append((t, front(t)))
                if len(pend) > 2:
                    back(*pend.pop(0))
            for item in pend:
                back(*item)

    nc.compile()
    return nc


def kernel(x, ctrs, Wv, Ov, k):
    from concourse.bass_utils import run_bass_kernel_spmd

    assert int(k) == K
    x = np.ascontiguousarray(np.asarray(x, dtype=np.float32))
    ctrs = np.ascontiguousarray(np.asarray(ctrs, dtype=np.float32))
    Wv = np.ascontiguousarray(np.asarray(Wv, dtype=np.float32))
    Ov = np.ascontiguousarray(np.asarray(Ov, dtype=np.float32))

    if "nc" not in _CACHE:
        _CACHE["nc"] = _build_program()
    nc = _CACHE["nc"]

    in_maps = [
        {"x": x[i * NS:(i + 1) * NS], "ctrs": ctrs, "Wv": Wv, "Ov": Ov}
        for i in range(NCORES)
    ]
    res = run_bass_kernel_spmd(nc, in_maps, core_ids=list(range(NCORES)))
    out = np.concatenate([res.results[i]["out"] for i in range(NCORES)], axis=0)
    return out.astype(np.float32)
